# revision 10
# baseline (speedup 1.0000x reference)
"""Causal attention head (B=4, T=2048, C=768, H=64) on 8 NeuronCores.

Data parallel: core 2b+h owns batch element b and query quarters
(0,3) for h=0, (1,2) for h=1 (zigzag causal load balance).

Per-core layout: xts columns are [A | B | F0 | F1] where A,B are the
core's own query quarters (A earlier) and F0,F1 hold the extra key
quarters each core needs:
  h=0: A=q0, B=q3, F0=q1, F1=q2
  h=1: A=q1, B=q2, F0=q0, F1=q0 (dup)
Every core runs the same 5 attention blocks of [512 q x 512 k]:
  D(A)       diagonal, triangular-packed    -> accA
  F(B, kA)   full                           -> accB
  D(B)       diagonal, triangular-packed    -> accB
  F(B, kF0)  full                           -> accB
  F(C, kF1)  full                           -> accC
where the C query group is built on-device as selA*Q_A + selB*Q_B from
a host-provided selector (h=0 selects B, h=1 selects A), making the
program uniform across cores with no masked-off waste.

Scores are computed transposed (keys on partitions, contraction 64).
Diagonal blocks are triangular-packed: key unit u only computes its
512-128u live query columns; a shared [128,512] lower-tri mask is
applied after exp.  Softmax denominators fall out of PV via a
ones-column appended to V.  V^T is transposed in pairs ([128,128] PE
transposes).  Outputs leave unnormalized as three [65, 512] f32
accumulators (numerator rows 0:64, denominator row 64); the host
divides, merges the C block into the right quarter, and transposes.
"""

from contextlib import ExitStack

import numpy as np

import concourse.bass as bass
import concourse.mybir as mybir
import concourse.tile as tile
from concourse import bacc
from concourse.masks import make_identity

FP16 = mybir.dt.float16
F32 = mybir.dt.float32

B, T, C, H = 4, 2048, 768, 64
QTR = 512
N_CORES = 8
SCALE = 1.0 / 8.0  # H ** -0.5

SLOT_A, SLOT_B, SLOT_F0, SLOT_F1 = 0, 1, 2, 3
QA, QB, QC = 0, QTR, 2 * QTR


def build_nc(dbg=False):
    nc = bacc.Bacc("TRN2", target_bir_lowering=False, debug=False,
                   num_devices=N_CORES)
    # xts is partition-major: xts[p, s*3072 + c*512 + q] = xT[c*128+p,
    # s*512+q], so each partition's per-slot data is one contiguous 6KB
    # run in DRAM (big DMA packets instead of 1KB lines).
    # Row-major [4*128, 3072]: slot s = rows s*128..s*128+127, so one
    # slot's 786KB is a single fully-contiguous DRAM block (sequential
    # HBM scan; the old column-sliced layout read 6KB runs at 48KB
    # stride, which halved effective HBM bandwidth).
    xts_e = nc.dram_tensor("xts", [4 * 128, 6 * QTR], FP16,
                           kind="ExternalInput")
    # wall packs [wkv (6*128) | wq (6*64) | sel (2)] so the weights ride in
    # one DMA (one fixed completion cost on the scalar HWDGE ring)
    wall_e = nc.dram_tensor("wall", [128, 6 * 128 + 6 * 64 + 2], FP16,
                            kind="ExternalInput")
    out_e = nc.dram_tensor("out", [65, 3 * QTR], FP16,
                           kind="ExternalOutput")
    if dbg:
        kt_d = nc.dram_tensor("kt_d", [64, T], FP16, kind="ExternalOutput")
        qt_d = nc.dram_tensor("qt_d", [64, 3 * QTR], FP16,
                              kind="ExternalOutput")
        v3_d = nc.dram_tensor("v3_d", [128, 16 * 65], FP16,
                              kind="ExternalOutput")

    with tile.TileContext(nc) as tc, ExitStack() as ctx:
        ep = ctx.enter_context

        const_p = ep(tc.tile_pool(name="const", bufs=1))
        xt_p = ep(tc.tile_pool(name="xt", bufs=1))
        w_p = ep(tc.tile_pool(name="w", bufs=1))
        big_p = ep(tc.tile_pool(name="big", bufs=1))
        eg_p = ep(tc.tile_pool(name="eg", bufs=4))
        o_p = ep(tc.tile_pool(name="o", bufs=3))
        kv_psp = ep(tc.tile_pool(name="kv_ps", bufs=1, space="PSUM"))
        q_psp = ep(tc.tile_pool(name="q_ps", bufs=1, space="PSUM"))
        sg_psp = ep(tc.tile_pool(name="sg_ps", bufs=2, space="PSUM"))
        acc_psp = ep(tc.tile_pool(name="acc_ps", bufs=1, space="PSUM"))
        vt_psp = ep(tc.tile_pool(name="vt_ps", bufs=1, space="PSUM"))

        # ---- input DMAs: weights, then x^T whole slots ------------------
        xt = xt_p.tile([128, 4, 6, QTR], FP16)
        wall = w_p.tile([128, 6 * 128 + 6 * 64 + 2], FP16)
        wkv = wall[:, 0:6 * 128]
        wq = wall[:, 6 * 128:6 * 128 + 6 * 64]
        sel16 = wall[:, 6 * 128 + 6 * 64:]
        sel = w_p.tile([128, 2], F32)

        def xt_dma(eng, s):
            # one whole slot per DMA, one contiguous 786KB DRAM block
            eng.dma_start(
                out=xt[:, s, :, :],
                in_=xts_e[s * 128:(s + 1) * 128, :].rearrange(
                    "p (n m) -> p n m", m=QTR))

        # Two HWDGE rings in parallel (sync + scalar), each FIFO: per-DMA
        # fixed completion costs overlap across rings, and slots are
        # ordered by consumption time (A -> B -> F0 -> F1).
        xt_dma(nc.sync, SLOT_A)
        nc.scalar.dma_start(out=wall[:, :], in_=wall_e[:, :])
        xt_dma(nc.scalar, SLOT_B)
        xt_dma(nc.sync, SLOT_F0)
        xt_dma(nc.scalar, SLOT_F1)
        nc.gpsimd.tensor_copy(sel[:, :], sel16[:, :])

        # ---- constants on gpsimd + exp table warm ----------------------
        mtri = const_p.tile([128, QTR], FP16)
        nc.gpsimd.memset(mtri[:, :], 1.0)
        warm = const_p.tile([128, 1], FP16)
        nc.scalar.activation(warm[:, :], mtri[:, 0:1],
                             mybir.ActivationFunctionType.Exp, scale=1.0)
        ident = const_p.tile([128, 128], FP16)
        make_identity(nc, ident[:, :])
        # mtri[s, q] = 1 iff q >= s (shared lower-tri mask, prefix-sliced)
        nc.gpsimd.affine_select(
            out=mtri[:, :], in_=mtri[:, :],
            compare_op=mybir.AluOpType.is_ge, fill=0.0,
            base=0, channel_multiplier=-1, pattern=[[1, QTR]])

        # V pair staging [Vt_{2m} ; Vt_{2m+1}] and V tiles with ones col
        vt2 = const_p.tile([128, 8, 128], FP16)
        v3 = big_p.tile([128, 16, 65], FP16)
        nc.gpsimd.memset(v3[:, :, 64:65], 1.0)

        kt = big_p.tile([64, T], FP16)
        qt = big_p.tile([64, 3 * QTR], FP16)  # [Q_A | Q_B | Q_C]
        fsrc = big_p.tile([128, 512], FP16)
        nc.vector.memset(fsrc[:, :], 0.0)

        # ---- helpers ---------------------------------------------------
        def kv_mm(kv, s, c):
            nc.tensor.matmul(kv[:, :], wkv[:, c * 128:(c + 1) * 128],
                             xt[:, s, c, :],
                             start=(c == 0), stop=(c == 5))

        def q_mm(q_ps, s, c):
            r0 = 0 if s == SLOT_A else 64
            nc.tensor.matmul(q_ps[r0:r0 + 64, :],
                             wq[:, c * 64:(c + 1) * 64],
                             xt[:, s, c, :], start=(c == 0), stop=(c == 5),
                             skip_group_check=True)

        def kv_evac(s, kv):
            # A/B evacs lean on ACT (idle before the exp stream starts);
            # F0/F1 evacs lean on DVE (ACT is saturated by exps then).
            js = slice(s * QTR, (s + 1) * QTR)
            if s in (SLOT_A, SLOT_B):
                nc.scalar.copy(kt[:, js], kv[0:64, :])
            else:
                nc.vector.tensor_copy(kt[:, js], kv[0:64, :])
            for j in range(4):
                m = s * 2 + j // 2
                r0 = (j % 2) * 64
                if (j % 2 == 0) == (s in (SLOT_A, SLOT_B)):
                    nc.vector.tensor_copy(vt2[r0:r0 + 64, m, :],
                                          kv[64:128, j * 128:(j + 1) * 128])
                else:
                    nc.scalar.copy(vt2[r0:r0 + 64, m, :],
                                   kv[64:128, j * 128:(j + 1) * 128])

        def q_evac(s, q_ps):
            r0, qb = (0, QA) if s == SLOT_A else (64, QB)
            nc.vector.tensor_copy(qt[:, qb:qb + QTR], q_ps[r0:r0 + 64, :])

        def v_pair_tr(m):
            """vt2[m] [128,128] -> V blocks 2m, 2m+1 via one PE transpose."""
            vp = vt_psp.tile([128, 128], FP16, tag="vp", name="vp")
            nc.tensor.transpose(vp[:, :], vt2[:, m, :], ident[:, :])
            nc.vector.tensor_copy(v3[:, 2 * m, 0:64], vp[:, 0:64])
            nc.vector.tensor_copy(v3[:, 2 * m + 1, 0:64], vp[:, 64:128])

        accs = {}
        n_pv = {"A": 4, "B": 12, "C": 4}
        pv_done = {"A": 0, "B": 0, "C": 0}

        def get_acc(a):
            if a not in accs:
                # accC takes the vtp bank (dead after the last transpose)
                # so F(C) accumulation overlaps accB's finalize
                pool, tag = (vt_psp, "vp") if a == "C" else (acc_psp, "acc")
                accs[a] = pool.tile([65, QTR], F32, tag=tag, name=f"acc{a}")
            return accs[a]

        def pv(a, u, eg, e0, n, q0):
            acc = get_acc(a)
            nc.tensor.matmul(acc[:, q0:q0 + n], v3[:, u, :],
                             eg[:, e0:e0 + n],
                             start=(pv_done[a] == 0),
                             stop=(pv_done[a] == n_pv[a] - 1))
            pv_done[a] += 1

        def full_group(u0, qb):
            """Scores+exp for full units u0, u0+1, N=512 each."""
            sg = sg_psp.tile([128, 2 * QTR], F32, tag="sg", name="sg")
            for i in range(2):
                u = u0 + i
                nc.tensor.matmul(sg[:, i * QTR:(i + 1) * QTR],
                                 kt[:, u * 128:(u + 1) * 128],
                                 qt[:, qb:qb + QTR], start=True, stop=True)
            eg = eg_p.tile([128, 2 * QTR], FP16, tag="eg", name="eg")
            nc.scalar.activation(eg[:, :], sg[:, :],
                                 mybir.ActivationFunctionType.Exp,
                                 scale=SCALE)
            return eg

        def full_pv(a, u0, eg):
            pv(a, u0, eg, 0, QTR, 0)
            pv(a, u0 + 1, eg, QTR, QTR, 0)

        def diag_g1(u0, qb):
            """Diag units u0 (N=512) + u0+1 (N=384), packed [0:896]."""
            sg = sg_psp.tile([128, 2 * QTR], F32, tag="sg", name="sg")
            nc.tensor.matmul(sg[:, 0:512], kt[:, u0 * 128:u0 * 128 + 128],
                             qt[:, qb:qb + 512], start=True, stop=True)
            nc.tensor.matmul(sg[:, 512:896],
                             kt[:, (u0 + 1) * 128:(u0 + 2) * 128],
                             qt[:, qb + 128:qb + 512], start=True, stop=True)
            eg = eg_p.tile([128, 2 * QTR], FP16, tag="eg", name="eg")
            nc.scalar.activation(eg[:, 0:896], sg[:, 0:896],
                                 mybir.ActivationFunctionType.Exp,
                                 scale=SCALE)
            nc.vector.tensor_mul(eg[:, 0:512], eg[:, 0:512], mtri[:, 0:512])
            nc.vector.tensor_mul(eg[:, 512:896], eg[:, 512:896],
                                 mtri[:, 0:384])
            return eg

        def diag_g1_pv(a, u0, eg):
            pv(a, u0, eg, 0, 512, 0)
            pv(a, u0 + 1, eg, 512, 384, 128)

        def diag_g2(u0, qb):
            """Diag units u0+2 (N=256) + u0+3 (N=128), packed [0:384]."""
            sg = sg_psp.tile([128, 2 * QTR], F32, tag="sg", name="sg")
            nc.tensor.matmul(sg[:, 0:256],
                             kt[:, (u0 + 2) * 128:(u0 + 3) * 128],
                             qt[:, qb + 256:qb + 512], start=True, stop=True)
            nc.tensor.matmul(sg[:, 256:384],
                             kt[:, (u0 + 3) * 128:(u0 + 4) * 128],
                             qt[:, qb + 384:qb + 512], start=True, stop=True)
            eg = eg_p.tile([128, 2 * QTR], FP16, tag="eg", name="eg")
            nc.scalar.activation(eg[:, 0:384], sg[:, 0:384],
                                 mybir.ActivationFunctionType.Exp,
                                 scale=SCALE)
            nc.gpsimd.tensor_mul(eg[:, 0:256], eg[:, 0:256], mtri[:, 0:256])
            nc.gpsimd.tensor_mul(eg[:, 256:384], eg[:, 256:384],
                                 mtri[:, 0:128])
            return eg

        def diag_g2_pv(a, u0, eg):
            pv(a, u0 + 2, eg, 0, 256, 256)
            pv(a, u0 + 3, eg, 256, 128, 384)

        ostg = o_p.tile([65, 3 * QTR], FP16)

        def finalize(a, i):
            nc.vector.tensor_copy(ostg[:, i * QTR:(i + 1) * QTR],
                                  accs[a][:, :])
            if i == 1:
                nc.sync.dma_start(out=out_e[:, 0:2 * QTR],
                                  in_=ostg[:, 0:2 * QTR])
            elif i == 2:
                nc.sync.dma_start(out=out_e[:, 2 * QTR:3 * QTR],
                                  in_=ostg[:, 2 * QTR:3 * QTR])

        # ================= schedule =====================================
        # PE warm-up / keep-warm fillers: the HAM clock gate throttles the
        # PE to 1.2 GHz unless it sees ~3.4us of sustained activity, and
        # any DMA-wait gap re-throttles it.  Cheap dummy matmuls bridge
        # the input-paced stalls so real work runs at 2.4 GHz.
        q_ps = q_psp.tile([128, QTR], F32, tag="q", name="q_ps")

        def fill(n, tgt=None, width=512):
            # dummy MMs confined to rows 0:64 of a PSUM region that holds
            # no live accumulation (q_ps rows 0:64 are dead once Q_A is
            # evacuated; Q_B accumulates in rows 64:128)
            t = q_ps if tgt is None else tgt
            for _ in range(n):
                nc.tensor.matmul(t[0:64, 0:width], fsrc[:, 0:64],
                                 fsrc[:, 0:width], start=True, stop=True,
                                 skip_group_check=True)

        fill(6)

        kvA = kv_psp.tile([128, QTR], F32, tag="kv", name="kvA")
        for c in range(6):
            kv_mm(kvA, SLOT_A, c)
            q_mm(q_ps, SLOT_A, c)
        kv_evac(SLOT_A, kvA)
        q_evac(SLOT_A, q_ps)

        egA1 = diag_g1(0, QA)
        v_pair_tr(0)
        fill(1)
        kvB = kv_psp.tile([128, QTR], F32, tag="kv", name="kvB")
        for c in range(3):
            kv_mm(kvB, SLOT_B, c)
            q_mm(q_ps, SLOT_B, c)
        v_pair_tr(1)
        egA2 = diag_g2(0, QA)
        diag_g1_pv("A", 0, egA1)
        for c in range(3, 6):
            kv_mm(kvB, SLOT_B, c)
            q_mm(q_ps, SLOT_B, c)
        kv_evac(SLOT_B, kvB)
        q_evac(SLOT_B, q_ps)
        diag_g2_pv("A", 0, egA2)
        finalize("A", 0)

        # Q_C = selA*Q_A + selB*Q_B
        nc.vector.tensor_scalar(out=qt[:, QC:QC + QTR],
                                in0=qt[:, QB:QB + QTR],
                                scalar1=sel[0:64, 1:2], scalar2=None,
                                op0=mybir.AluOpType.mult)
        nc.vector.scalar_tensor_tensor(out=qt[:, QC:QC + QTR],
                                       in0=qt[:, QA:QA + QTR],
                                       scalar=sel[0:64, 0:1],
                                       in1=qt[:, QC:QC + QTR],
                                       op0=mybir.AluOpType.mult,
                                       op1=mybir.AluOpType.add)

        # F(B, kA) / F0 proj / slot-B transposes, pipelined depth 2
        egBA1 = full_group(0, QB)
        fill(1)
        kvF0 = kv_psp.tile([128, QTR], F32, tag="kv", name="kvF0")
        for c in range(3):
            kv_mm(kvF0, SLOT_F0, c)
        v_pair_tr(2)
        egBA2 = full_group(2, QB)
        full_pv("B", 0, egBA1)
        for c in range(3, 6):
            kv_mm(kvF0, SLOT_F0, c)
        v_pair_tr(3)
        egB1 = diag_g1(4, QB)
        full_pv("B", 2, egBA2)
        kv_evac(SLOT_F0, kvF0)

        # D(B) / F1 proj / slot-F0 transposes
        fill(1)
        kvF1 = q_psp.tile([128, QTR], F32, tag="q", name="kvF1")
        for c in range(3):
            kv_mm(kvF1, SLOT_F1, c)
        v_pair_tr(4)
        egB2 = diag_g2(4, QB)
        diag_g1_pv("B", 4, egB1)
        for c in range(3, 6):
            kv_mm(kvF1, SLOT_F1, c)
        v_pair_tr(5)
        diag_g2_pv("B", 4, egB2)
        kv_evac(SLOT_F1, kvF1)

        # F(B, kF0), F(C, kF1), depth-2 pipelined tail
        fillC = kv_psp.tile([128, QTR], F32, tag="kv", name="fillC")
        egF01 = full_group(8, QB)
        v_pair_tr(6)
        fill(1, fillC, width=256)
        egF02 = full_group(10, QB)
        full_pv("B", 8, egF01)
        v_pair_tr(7)
        fill(1, fillC, width=256)
        egF11 = full_group(12, QC)
        full_pv("B", 10, egF02)
        finalize("B", 1)
        sgF12 = sg_psp.tile([128, 2 * QTR], F32, tag="sg", name="sgF12")
        for i in range(2):
            nc.tensor.matmul(sgF12[:, i * QTR:(i + 1) * QTR],
                             kt[:, (14 + i) * 128:(15 + i) * 128],
                             qt[:, QC:QC + QTR], start=True, stop=True)
        full_pv("C", 12, egF11)
        egF12 = eg_p.tile([128, 2 * QTR], FP16, tag="eg", name="egF12")
        nc.scalar.activation(egF12[:, 0:QTR], sgF12[:, 0:QTR],
                             mybir.ActivationFunctionType.Exp, scale=SCALE)
        pv("C", 14, egF12, 0, QTR, 0)
        nc.scalar.activation(egF12[:, QTR:2 * QTR], sgF12[:, QTR:2 * QTR],
                             mybir.ActivationFunctionType.Exp, scale=SCALE)
        pv("C", 15, egF12, QTR, QTR, 0)
        finalize("C", 2)

        if dbg:
            nc.sync.dma_start(out=kt_d[:, :], in_=kt[:, :])
            nc.sync.dma_start(out=qt_d[:, :], in_=qt[:, :])
            nc.sync.dma_start(
                out=v3_d[:, :],
                in_=v3[:, :, :].rearrange("p a b -> p (a b)"))

    nc.compile()
    return nc


# ---------------- host-side shard / unshard ---------------------------

def _tile_weights(w, m):
    """[C, m] -> partition-major [128, 6*m] so the DMA is contiguous."""
    return np.ascontiguousarray(
        w.astype(np.float16).reshape(6, 128, m).transpose(1, 0, 2)
        .reshape(128, 6 * m))


def shard_inputs(x, Wq, Wk, Wv):
    """Full inputs -> list of 8 per-core input dicts."""
    wkv = _tile_weights(np.concatenate([Wk, Wv], axis=1), 128)
    wq16 = _tile_weights(Wq, 64)
    in_maps = []
    for b in range(B):
        xT = np.ascontiguousarray(x[b].astype(np.float16).T)  # [C, T]
        q = [xT[:, i * QTR:(i + 1) * QTR] for i in range(4)]
        for h in range(2):
            if h == 0:
                xts = np.concatenate([q[0], q[3], q[1], q[2]], axis=1)
                sa, sb = 0.0, 1.0
            else:
                xts = np.concatenate([q[1], q[2], q[0], q[0]], axis=1)
                sa, sb = 1.0, 0.0
            # [768, 2048] -> slot-major rows [4*128, 6*512]: slot s rows
            # are partition-major [128, chunks*cols], contiguous per slot
            xts = (xts.reshape(6, 128, 4, QTR).transpose(2, 1, 0, 3)
                   .reshape(4 * 128, 6 * QTR))
            sel = np.zeros((128, 2), np.float16)
            sel[:, 0] = sa
            sel[:, 1] = sb
            wall = np.concatenate([wkv, wq16, sel], axis=1)
            in_maps.append({
                "xts": np.ascontiguousarray(xts),
                "wall": np.ascontiguousarray(wall),
            })
    return in_maps


def unshard_outputs(results):
    """List of 8 per-core result dicts -> full [B, T, H] float32."""
    out = np.zeros((B, T, H), dtype=np.float32)
    for b in range(B):
        for h in range(2):
            raw = results[2 * b + h]["out"].astype(np.float32)
            accA = raw[:, 0:QTR]
            accB = raw[:, QTR:2 * QTR]
            accC = raw[:, 2 * QTR:3 * QTR]
            if h == 0:
                oA, oB = accA, accB + accC
                qa, qb = 0, 3
            else:
                oA, oB = accA + accC, accB
                qa, qb = 1, 2
            out[b, qa * QTR:(qa + 1) * QTR] = (oA[0:64] / oA[64:65]).T
            out[b, qb * QTR:(qb + 1) * QTR] = (oB[0:64] / oB[64:65]).T
    return out


# ---------------- harness entrypoint ----------------------------------

_NC_CACHE = []


def kernel(x, Wq, Wk, Wv):
    """Full inputs -> full [B, T, H] float32 output, computed on 8 cores."""
    from concourse.bass_utils import run_bass_kernel_spmd

    x = np.asarray(x); Wq = np.asarray(Wq)
    Wk = np.asarray(Wk); Wv = np.asarray(Wv)
    in_maps = shard_inputs(x, Wq, Wk, Wv)
    if not _NC_CACHE:
        _NC_CACHE.append(build_nc())
    nc = _NC_CACHE[0]
    res = run_bass_kernel_spmd(nc, in_maps, core_ids=list(range(N_CORES)))
    return unshard_outputs(res.results)



# revision 13
# speedup vs baseline: 1.1695x; 1.1695x over previous
"""Causal attention head (B=4, T=2048, C=768, H=64) on 8 NeuronCores.

Data parallel: core 2b+h owns batch element b and query quarters
(0,3) for h=0, (1,2) for h=1 (zigzag causal load balance).

Per-core layout: xts columns are [A | B | F0 | F1] where A,B are the
core's own query quarters (A earlier) and F0,F1 hold the extra key
quarters each core needs:
  h=0: A=q0, B=q3, F0=q1, F1=q2
  h=1: A=q1, B=q2, F0=q0, F1=q0 (dup)
Every core runs the same 5 attention blocks of [512 q x 512 k]:
  D(A)       diagonal, triangular-packed    -> accA
  F(B, kA)   full                           -> accB
  D(B)       diagonal, triangular-packed    -> accB
  F(B, kF0)  full                           -> accB
  F(C, kF1)  full                           -> accC
where the C query group is built on-device as selA*Q_A + selB*Q_B from
a host-provided selector (h=0 selects B, h=1 selects A), making the
program uniform across cores with no masked-off waste.

Scores are computed transposed (keys on partitions, contraction 64).
Diagonal blocks are triangular-packed: key unit u only computes its
512-128u live query columns; a shared [128,512] lower-tri mask is
applied after exp.  Softmax denominators fall out of PV via a
ones-column appended to V.  V^T is transposed in pairs ([128,128] PE
transposes).  Outputs leave unnormalized as three [65, 512] f32
accumulators (numerator rows 0:64, denominator row 64); the host
divides, merges the C block into the right quarter, and transposes.
"""

from contextlib import ExitStack

import numpy as np

import concourse.bass as bass
import concourse.mybir as mybir
import concourse.tile as tile
from concourse import bacc
from concourse.masks import make_identity

FP16 = mybir.dt.float16
F32 = mybir.dt.float32

B, T, C, H = 4, 2048, 768, 64
QTR = 512
N_CORES = 8
SCALE = 1.0 / 8.0  # H ** -0.5

SLOT_A, SLOT_B, SLOT_F0, SLOT_F1 = 0, 1, 2, 3
QA, QB, QC = 0, QTR, 2 * QTR


def build_nc(dbg=False):
    nc = bacc.Bacc("TRN2", target_bir_lowering=False, debug=False,
                   num_devices=N_CORES)
    # xts is partition-major: xts[p, s*3072 + c*512 + q] = xT[c*128+p,
    # s*512+q], so each partition's per-slot data is one contiguous 6KB
    # run in DRAM (big DMA packets instead of 1KB lines).
    # Row-major [4*128, 3072]: slot s = rows s*128..s*128+127, so one
    # slot's 786KB is a single fully-contiguous DRAM block (sequential
    # HBM scan; the old column-sliced layout read 6KB runs at 48KB
    # stride, which halved effective HBM bandwidth).
    xts_e = nc.dram_tensor("xts", [4 * 128, 6 * QTR], FP16,
                           kind="ExternalInput")
    # wall packs [wkv (6*128) | wq (6*64) | sel (2)] so the weights ride in
    # one DMA (one fixed completion cost on the scalar HWDGE ring)
    wall_e = nc.dram_tensor("wall", [128, 6 * 128 + 6 * 64 + 2], FP16,
                            kind="ExternalInput")
    out_e = nc.dram_tensor("out", [65, 3 * QTR], FP16,
                           kind="ExternalOutput")
    if dbg:
        kt_d = nc.dram_tensor("kt_d", [64, T], FP16, kind="ExternalOutput")
        qt_d = nc.dram_tensor("qt_d", [64, 3 * QTR], FP16,
                              kind="ExternalOutput")
        v3_d = nc.dram_tensor("v3_d", [128, 16 * 65], FP16,
                              kind="ExternalOutput")

    with tile.TileContext(nc) as tc, ExitStack() as ctx:
        ep = ctx.enter_context

        const_p = ep(tc.tile_pool(name="const", bufs=1))
        xt_p = ep(tc.tile_pool(name="xt", bufs=1))
        w_p = ep(tc.tile_pool(name="w", bufs=1))
        big_p = ep(tc.tile_pool(name="big", bufs=1))
        eg_p = ep(tc.tile_pool(name="eg", bufs=4))
        o_p = ep(tc.tile_pool(name="o", bufs=3))
        kv_psp = ep(tc.tile_pool(name="kv_ps", bufs=1, space="PSUM"))
        q_psp = ep(tc.tile_pool(name="q_ps", bufs=1, space="PSUM"))
        sg_psp = ep(tc.tile_pool(name="sg_ps", bufs=2, space="PSUM"))
        acc_psp = ep(tc.tile_pool(name="acc_ps", bufs=1, space="PSUM"))
        vt_psp = ep(tc.tile_pool(name="vt_ps", bufs=1, space="PSUM"))

        # ---- input DMAs: weights, then x^T whole slots ------------------
        xt = xt_p.tile([128, 4, 6, QTR], FP16)
        wall = w_p.tile([128, 6 * 128 + 6 * 64 + 2], FP16)
        wkv = wall[:, 0:6 * 128]
        wq = wall[:, 6 * 128:6 * 128 + 6 * 64]
        sel16 = wall[:, 6 * 128 + 6 * 64:]
        sel = w_p.tile([128, 2], F32)

        def xt_dma(eng, s, c0, c1):
            # 2-chunk granules: 262KB DMAs with 2KB contiguous runs per
            # partition keep the sync ring at ~400GB/s (1KB runs drop to
            # ~165GB/s); deep FIFO queue pipelines them back to back
            eng.dma_start(
                out=xt[:, s, c0:c1, :],
                in_=xts_e[s * 128:(s + 1) * 128, c0 * QTR:c1 * QTR]
                .rearrange("p (n m) -> p n m", m=QTR))

        # All x granules on the sync ring in consumption order; the first
        # DMA pays ~4.6us fixed latency, the rest stream at line rate.
        # wall rides the scalar ring concurrently (same first-latency).
        nc.scalar.dma_start(out=wall[:, :], in_=wall_e[:, :])
        for s in (SLOT_A, SLOT_B, SLOT_F0, SLOT_F1):
            for c0 in (0, 2, 4):
                xt_dma(nc.sync, s, c0, c0 + 2)

        # ---- constants on gpsimd + exp table warm ----------------------
        mtri = const_p.tile([128, QTR], FP16)
        nc.gpsimd.memset(mtri[:, :], 1.0)
        warm = const_p.tile([128, 1], FP16)
        nc.scalar.activation(warm[:, :], mtri[:, 0:1],
                             mybir.ActivationFunctionType.Exp, scale=1.0)
        ident = const_p.tile([128, 128], FP16)
        make_identity(nc, ident[:, :])
        # mtri[s, q] = 1 iff q >= s (shared lower-tri mask, prefix-sliced)
        nc.gpsimd.affine_select(
            out=mtri[:, :], in_=mtri[:, :],
            compare_op=mybir.AluOpType.is_ge, fill=0.0,
            base=0, channel_multiplier=-1, pattern=[[1, QTR]])
        nc.gpsimd.tensor_copy(sel[:, :], sel16[:, :])

        # V pair staging [Vt_{2m} ; Vt_{2m+1}] and V tiles with ones col
        vt2 = const_p.tile([128, 8, 128], FP16)
        v3 = big_p.tile([128, 16, 65], FP16)
        nc.gpsimd.memset(v3[:, :, 64:65], 1.0)

        kt = big_p.tile([64, T], FP16)
        qt = big_p.tile([64, 3 * QTR], FP16)  # [Q_A | Q_B | Q_C]
        fsrc = big_p.tile([128, 512], FP16)
        nc.vector.memset(fsrc[:, :], 0.0)

        # ---- helpers ---------------------------------------------------
        def kv_mm(kv, s, c):
            nc.tensor.matmul(kv[:, :], wkv[:, c * 128:(c + 1) * 128],
                             xt[:, s, c, :],
                             start=(c == 0), stop=(c == 5))

        def q_mm(q_ps, s, c):
            r0 = 0 if s == SLOT_A else 64
            nc.tensor.matmul(q_ps[r0:r0 + 64, :],
                             wq[:, c * 64:(c + 1) * 64],
                             xt[:, s, c, :], start=(c == 0), stop=(c == 5),
                             skip_group_check=True)

        def kv_evac(s, kv):
            # A/B evacs lean on ACT (idle before the exp stream starts);
            # F0/F1 evacs lean on DVE (ACT is saturated by exps then).
            js = slice(s * QTR, (s + 1) * QTR)
            if s in (SLOT_A, SLOT_B):
                nc.scalar.copy(kt[:, js], kv[0:64, :])
            else:
                nc.vector.tensor_copy(kt[:, js], kv[0:64, :])
            for j in range(4):
                m = s * 2 + j // 2
                r0 = (j % 2) * 64
                if (j % 2 == 0) == (s in (SLOT_A, SLOT_B)):
                    nc.vector.tensor_copy(vt2[r0:r0 + 64, m, :],
                                          kv[64:128, j * 128:(j + 1) * 128])
                else:
                    nc.scalar.copy(vt2[r0:r0 + 64, m, :],
                                   kv[64:128, j * 128:(j + 1) * 128])

        def q_evac(s, q_ps):
            r0, qb = (0, QA) if s == SLOT_A else (64, QB)
            nc.vector.tensor_copy(qt[:, qb:qb + QTR], q_ps[r0:r0 + 64, :])

        def v_pair_tr(m):
            """vt2[m] [128,128] -> V blocks 2m, 2m+1 via one PE transpose."""
            vp = vt_psp.tile([128, 128], FP16, tag="vp", name="vp")
            nc.tensor.transpose(vp[:, :], vt2[:, m, :], ident[:, :])
            nc.vector.tensor_copy(v3[:, 2 * m, 0:64], vp[:, 0:64])
            nc.vector.tensor_copy(v3[:, 2 * m + 1, 0:64], vp[:, 64:128])

        accs = {}
        n_pv = {"A": 4, "B": 12, "C": 4}
        pv_done = {"A": 0, "B": 0, "C": 0}

        def get_acc(a):
            if a not in accs:
                # accC takes the vtp bank (dead after the last transpose)
                # so F(C) accumulation overlaps accB's finalize
                pool, tag = (vt_psp, "vp") if a == "C" else (acc_psp, "acc")
                accs[a] = pool.tile([65, QTR], F32, tag=tag, name=f"acc{a}")
            return accs[a]

        def pv(a, u, eg, e0, n, q0):
            acc = get_acc(a)
            nc.tensor.matmul(acc[:, q0:q0 + n], v3[:, u, :],
                             eg[:, e0:e0 + n],
                             start=(pv_done[a] == 0),
                             stop=(pv_done[a] == n_pv[a] - 1))
            pv_done[a] += 1

        def full_group(u0, qb):
            """Scores+exp for full units u0, u0+1, N=512 each."""
            sg = sg_psp.tile([128, 2 * QTR], F32, tag="sg", name="sg")
            for i in range(2):
                u = u0 + i
                nc.tensor.matmul(sg[:, i * QTR:(i + 1) * QTR],
                                 kt[:, u * 128:(u + 1) * 128],
                                 qt[:, qb:qb + QTR], start=True, stop=True)
            eg = eg_p.tile([128, 2 * QTR], FP16, tag="eg", name="eg")
            nc.scalar.activation(eg[:, :], sg[:, :],
                                 mybir.ActivationFunctionType.Exp,
                                 scale=SCALE)
            return eg

        def full_pv(a, u0, eg):
            pv(a, u0, eg, 0, QTR, 0)
            pv(a, u0 + 1, eg, QTR, QTR, 0)

        def diag_g1(u0, qb):
            """Diag units u0 (N=512) + u0+1 (N=384), packed [0:896]."""
            sg = sg_psp.tile([128, 2 * QTR], F32, tag="sg", name="sg")
            nc.tensor.matmul(sg[:, 0:512], kt[:, u0 * 128:u0 * 128 + 128],
                             qt[:, qb:qb + 512], start=True, stop=True)
            nc.tensor.matmul(sg[:, 512:896],
                             kt[:, (u0 + 1) * 128:(u0 + 2) * 128],
                             qt[:, qb + 128:qb + 512], start=True, stop=True)
            eg = eg_p.tile([128, 2 * QTR], FP16, tag="eg", name="eg")
            nc.scalar.activation(eg[:, 0:896], sg[:, 0:896],
                                 mybir.ActivationFunctionType.Exp,
                                 scale=SCALE)
            nc.vector.tensor_mul(eg[:, 0:512], eg[:, 0:512], mtri[:, 0:512])
            nc.vector.tensor_mul(eg[:, 512:896], eg[:, 512:896],
                                 mtri[:, 0:384])
            return eg

        def diag_g1_pv(a, u0, eg):
            pv(a, u0, eg, 0, 512, 0)
            pv(a, u0 + 1, eg, 512, 384, 128)

        def diag_g2(u0, qb):
            """Diag units u0+2 (N=256) + u0+3 (N=128), packed [0:384]."""
            sg = sg_psp.tile([128, 2 * QTR], F32, tag="sg", name="sg")
            nc.tensor.matmul(sg[:, 0:256],
                             kt[:, (u0 + 2) * 128:(u0 + 3) * 128],
                             qt[:, qb + 256:qb + 512], start=True, stop=True)
            nc.tensor.matmul(sg[:, 256:384],
                             kt[:, (u0 + 3) * 128:(u0 + 4) * 128],
                             qt[:, qb + 384:qb + 512], start=True, stop=True)
            eg = eg_p.tile([128, 2 * QTR], FP16, tag="eg", name="eg")
            nc.scalar.activation(eg[:, 0:384], sg[:, 0:384],
                                 mybir.ActivationFunctionType.Exp,
                                 scale=SCALE)
            nc.gpsimd.tensor_mul(eg[:, 0:256], eg[:, 0:256], mtri[:, 0:256])
            nc.gpsimd.tensor_mul(eg[:, 256:384], eg[:, 256:384],
                                 mtri[:, 0:128])
            return eg

        def diag_g2_pv(a, u0, eg):
            pv(a, u0 + 2, eg, 0, 256, 256)
            pv(a, u0 + 3, eg, 256, 128, 384)

        ostg = o_p.tile([65, 3 * QTR], FP16)

        def finalize(a, i):
            nc.vector.tensor_copy(ostg[:, i * QTR:(i + 1) * QTR],
                                  accs[a][:, :])
            if i == 1:
                nc.sync.dma_start(out=out_e[:, 0:2 * QTR],
                                  in_=ostg[:, 0:2 * QTR])
            elif i == 2:
                nc.sync.dma_start(out=out_e[:, 2 * QTR:3 * QTR],
                                  in_=ostg[:, 2 * QTR:3 * QTR])

        # ================= schedule =====================================
        # PE warm-up / keep-warm fillers: the HAM clock gate throttles the
        # PE to 1.2 GHz unless it sees ~3.4us of sustained activity, and
        # any DMA-wait gap re-throttles it.  Cheap dummy matmuls bridge
        # the input-paced stalls so real work runs at 2.4 GHz.
        q_ps = q_psp.tile([128, QTR], F32, tag="q", name="q_ps")

        def fill(n, tgt=None, width=512):
            # dummy MMs confined to rows 0:64 of a PSUM region that holds
            # no live accumulation (q_ps rows 0:64 are dead once Q_A is
            # evacuated; Q_B accumulates in rows 64:128)
            t = q_ps if tgt is None else tgt
            for _ in range(n):
                nc.tensor.matmul(t[0:64, 0:width], fsrc[:, 0:64],
                                 fsrc[:, 0:width], start=True, stop=True,
                                 skip_group_check=True)

        fill(10)

        kvA = kv_psp.tile([128, QTR], F32, tag="kv", name="kvA")
        for c in range(6):
            kv_mm(kvA, SLOT_A, c)
            q_mm(q_ps, SLOT_A, c)
        kv_evac(SLOT_A, kvA)
        q_evac(SLOT_A, q_ps)

        egA1 = diag_g1(0, QA)
        v_pair_tr(0)
        fill(1)
        kvB = kv_psp.tile([128, QTR], F32, tag="kv", name="kvB")
        for c in range(3):
            kv_mm(kvB, SLOT_B, c)
            q_mm(q_ps, SLOT_B, c)
        v_pair_tr(1)
        egA2 = diag_g2(0, QA)
        diag_g1_pv("A", 0, egA1)
        for c in range(3, 6):
            kv_mm(kvB, SLOT_B, c)
            q_mm(q_ps, SLOT_B, c)
        kv_evac(SLOT_B, kvB)
        q_evac(SLOT_B, q_ps)
        diag_g2_pv("A", 0, egA2)
        finalize("A", 0)

        # Q_C = selA*Q_A + selB*Q_B
        nc.vector.tensor_scalar(out=qt[:, QC:QC + QTR],
                                in0=qt[:, QB:QB + QTR],
                                scalar1=sel[0:64, 1:2], scalar2=None,
                                op0=mybir.AluOpType.mult)
        nc.vector.scalar_tensor_tensor(out=qt[:, QC:QC + QTR],
                                       in0=qt[:, QA:QA + QTR],
                                       scalar=sel[0:64, 0:1],
                                       in1=qt[:, QC:QC + QTR],
                                       op0=mybir.AluOpType.mult,
                                       op1=mybir.AluOpType.add)

        # F(B, kA) / F0 proj / slot-B transposes, pipelined depth 2
        egBA1 = full_group(0, QB)
        fill(1)
        kvF0 = kv_psp.tile([128, QTR], F32, tag="kv", name="kvF0")
        for c in range(3):
            kv_mm(kvF0, SLOT_F0, c)
        v_pair_tr(2)
        egBA2 = full_group(2, QB)
        full_pv("B", 0, egBA1)
        for c in range(3, 6):
            kv_mm(kvF0, SLOT_F0, c)
        v_pair_tr(3)
        egB1 = diag_g1(4, QB)
        full_pv("B", 2, egBA2)
        kv_evac(SLOT_F0, kvF0)

        # D(B) / F1 proj / slot-F0 transposes
        fill(1)
        kvF1 = q_psp.tile([128, QTR], F32, tag="q", name="kvF1")
        for c in range(3):
            kv_mm(kvF1, SLOT_F1, c)
        v_pair_tr(4)
        egB2 = diag_g2(4, QB)
        diag_g1_pv("B", 4, egB1)
        for c in range(3, 6):
            kv_mm(kvF1, SLOT_F1, c)
        v_pair_tr(5)
        diag_g2_pv("B", 4, egB2)
        kv_evac(SLOT_F1, kvF1)

        # F(B, kF0), F(C, kF1), depth-2 pipelined tail
        fillC = kv_psp.tile([128, QTR], F32, tag="kv", name="fillC")
        egF01 = full_group(8, QB)
        v_pair_tr(6)
        fill(1, fillC, width=256)
        egF02 = full_group(10, QB)
        full_pv("B", 8, egF01)
        v_pair_tr(7)
        fill(1, fillC, width=256)
        egF11 = full_group(12, QC)
        full_pv("B", 10, egF02)
        finalize("B", 1)
        sgF12 = sg_psp.tile([128, 2 * QTR], F32, tag="sg", name="sgF12")
        for i in range(2):
            nc.tensor.matmul(sgF12[:, i * QTR:(i + 1) * QTR],
                             kt[:, (14 + i) * 128:(15 + i) * 128],
                             qt[:, QC:QC + QTR], start=True, stop=True)
        full_pv("C", 12, egF11)
        egF12 = eg_p.tile([128, 2 * QTR], FP16, tag="eg", name="egF12")
        nc.scalar.activation(egF12[:, 0:QTR], sgF12[:, 0:QTR],
                             mybir.ActivationFunctionType.Exp, scale=SCALE)
        pv("C", 14, egF12, 0, QTR, 0)
        nc.scalar.activation(egF12[:, QTR:2 * QTR], sgF12[:, QTR:2 * QTR],
                             mybir.ActivationFunctionType.Exp, scale=SCALE)
        pv("C", 15, egF12, QTR, QTR, 0)
        finalize("C", 2)

        if dbg:
            nc.sync.dma_start(out=kt_d[:, :], in_=kt[:, :])
            nc.sync.dma_start(out=qt_d[:, :], in_=qt[:, :])
            nc.sync.dma_start(
                out=v3_d[:, :],
                in_=v3[:, :, :].rearrange("p a b -> p (a b)"))

    nc.compile()
    return nc


# ---------------- host-side shard / unshard ---------------------------

def _tile_weights(w, m):
    """[C, m] -> partition-major [128, 6*m] so the DMA is contiguous."""
    return np.ascontiguousarray(
        w.astype(np.float16).reshape(6, 128, m).transpose(1, 0, 2)
        .reshape(128, 6 * m))


def shard_inputs(x, Wq, Wk, Wv):
    """Full inputs -> list of 8 per-core input dicts."""
    wkv = _tile_weights(np.concatenate([Wk, Wv], axis=1), 128)
    wq16 = _tile_weights(Wq, 64)
    in_maps = []
    for b in range(B):
        xT = np.ascontiguousarray(x[b].astype(np.float16).T)  # [C, T]
        q = [xT[:, i * QTR:(i + 1) * QTR] for i in range(4)]
        for h in range(2):
            if h == 0:
                xts = np.concatenate([q[0], q[3], q[1], q[2]], axis=1)
                sa, sb = 0.0, 1.0
            else:
                xts = np.concatenate([q[1], q[2], q[0], q[0]], axis=1)
                sa, sb = 1.0, 0.0
            # [768, 2048] -> slot-major rows [4*128, 6*512]: slot s rows
            # are partition-major [128, chunks*cols], contiguous per slot
            xts = (xts.reshape(6, 128, 4, QTR).transpose(2, 1, 0, 3)
                   .reshape(4 * 128, 6 * QTR))
            sel = np.zeros((128, 2), np.float16)
            sel[:, 0] = sa
            sel[:, 1] = sb
            wall = np.concatenate([wkv, wq16, sel], axis=1)
            in_maps.append({
                "xts": np.ascontiguousarray(xts),
                "wall": np.ascontiguousarray(wall),
            })
    return in_maps


def unshard_outputs(results):
    """List of 8 per-core result dicts -> full [B, T, H] float32."""
    out = np.zeros((B, T, H), dtype=np.float32)
    for b in range(B):
        for h in range(2):
            raw = results[2 * b + h]["out"].astype(np.float32)
            accA = raw[:, 0:QTR]
            accB = raw[:, QTR:2 * QTR]
            accC = raw[:, 2 * QTR:3 * QTR]
            if h == 0:
                oA, oB = accA, accB + accC
                qa, qb = 0, 3
            else:
                oA, oB = accA + accC, accB
                qa, qb = 1, 2
            out[b, qa * QTR:(qa + 1) * QTR] = (oA[0:64] / oA[64:65]).T
            out[b, qb * QTR:(qb + 1) * QTR] = (oB[0:64] / oB[64:65]).T
    return out


# ---------------- harness entrypoint ----------------------------------

_NC_CACHE = []


def kernel(x, Wq, Wk, Wv):
    """Full inputs -> full [B, T, H] float32 output, computed on 8 cores."""
    from concourse.bass_utils import run_bass_kernel_spmd

    x = np.asarray(x); Wq = np.asarray(Wq)
    Wk = np.asarray(Wk); Wv = np.asarray(Wv)
    in_maps = shard_inputs(x, Wq, Wk, Wv)
    if not _NC_CACHE:
        _NC_CACHE.append(build_nc())
    nc = _NC_CACHE[0]
    res = run_bass_kernel_spmd(nc, in_maps, core_ids=list(range(N_CORES)))
    return unshard_outputs(res.results)

